# revision 1
# baseline (speedup 1.0000x reference)
"""Trainium2 Bass kernel for ragged subword mean pooling (nn_Bert).

Problem: out[b, j] = mean(bert_embedding[b, st_j:ed_j]) if (mask & ed>st) else 0
Shapes: bert_embedding [32, 1024, 768] f32, x_bert_offset [32, 768, 2] i32,
        x_mask [32, 768] i32 -> out [32, 768, 768] f32.

Strategy (pure data parallel, 4 batch rows per core on 8 cores):
  Spans are contiguous sorted segments, so per row the pooling is
  out = A.T @ E where A[s, j] = scale_j iff st_j <= s < ed_j
  (scale_j = valid/len folds the mean and mask directly into A).
  Each position s belongs to at most ONE word, so every A tile has at
  most one nonzero per partition row. The host ships just that
  (column, value) pair per position (~32KB/core) and the device
  reconstructs each [128, win] A window in a single fused DVE op
  against a constant column-index tile J:
      A[p, j] = (J[p, j] == idx_p) * val_p
  The contraction runs on the PE in float32r (full rate; values are
  rounded to ~tf32, rel err ~1e-4). PSUM is drained by plain scalar-
  engine copies. Only (m, k) tile pairs whose word/position ranges
  intersect are computed; the active-pair hull is derived on the host
  from the actual offsets (a superset is always correct since A is 0
  outside).
"""

import sys

if "/opt/trn_rl_repo" not in sys.path:
    sys.path.insert(0, "/opt/trn_rl_repo")

import numpy as np

B, S, W, D = 32, 1024, 768, 768
NCORES = 8
RPC = B // NCORES  # rows per core
KT = S // 128  # 8 k-tiles (positions)
MT = W // 128  # 6 m-tiles (words)

_CACHE = {}


def _active_pairs(st, ed):
    """Per row-slot r: hull of active k-tiles for each m-tile, and hull of
    active m-tiles for each k-tile, unioned over cores (the SPMD program is
    shared by all 8 cores). A superset only costs time, never correctness.
    """
    kl = []
    for r in range(RPC):
        per_m = []
        for m in range(MT):
            klo, khi = KT, 0
            for c in range(NCORES):
                b = c * RPC + r
                s0 = int(st[b, m * 128 : (m + 1) * 128].min())
                s1 = int(ed[b, m * 128 : (m + 1) * 128].max())
                if s1 > s0:
                    klo = min(klo, s0 // 128)
                    khi = max(khi, (s1 + 127) // 128)
            per_m.append((klo, khi) if khi > klo else None)
        kl.append(per_m)

    mw = []
    for r in range(RPC):
        per_k = []
        for k in range(KT):
            mlo, mhi = MT, 0
            for m in range(MT):
                if kl[r][m] and kl[r][m][0] <= k < kl[r][m][1]:
                    mlo = min(mlo, m)
                    mhi = max(mhi, m + 1)
            per_k.append((mlo, mhi) if mhi > mlo else None)
        mw.append(per_k)
    return kl, mw


def build_program(pairs, repeat=1, drain="act", io="ext", stage=3, nodma=False,
                  ebufs=7, abufs=8, psbufs=3, obufs=6, avbufs=2):
    """Build the SPMD Bass program (one program, run on all 8 cores)."""
    import concourse.tile as tile
    from concourse import bacc, mybir

    kl, mw = pairs
    f32 = mybir.dt.float32
    f32r = mybir.dt.float32r
    i32 = mybir.dt.int32
    AF = mybir.ActivationFunctionType
    OP = mybir.AluOpType

    nc = bacc.Bacc(
        "TRN2", target_bir_lowering=False, debug=False, num_devices=NCORES
    )

    E_in = nc.dram_tensor("E_in", [RPC, S, D], f32r, kind="ExternalInput").ap()
    # packed per (r, k): column 2*(r*KT+k) = one-hot column index within the
    # A window (or -1), column +1 = A value (scale of the word at that
    # position, 0 if masked/empty/uncovered)
    av_in = nc.dram_tensor("av_in", [128, RPC * KT * 2], f32, kind="ExternalInput").ap()
    if io == "ext":
        out = nc.dram_tensor("out", [RPC, W, D], f32, kind="ExternalOutput").ap()
        tok = None
    else:
        out = nc.dram_tensor("out_scratch", [RPC, W, D], f32).ap()
        tok = nc.dram_tensor("tok", [128, 16], f32, kind="ExternalOutput").ap()
    outdma = not nodma

    def win(r, k):
        if mw[r][k] is None:
            return None
        mlo, mhi = mw[r][k]
        return mlo * 128, (mhi - mlo) * 128

    awidth = 128
    for r in range(RPC):
        for k in range(KT):
            if mw[r][k]:
                awidth = max(awidth, (mw[r][k][1] - mw[r][k][0]) * 128)

    any_empty_m = any(kl[r][m] is None for r in range(RPC) for m in range(MT))

    with tile.TileContext(nc) as tc:
        with (
            tc.tile_pool(name="const", bufs=1) as cpool,
            tc.tile_pool(name="E", bufs=ebufs) as epool,
            tc.tile_pool(name="bc", bufs=avbufs) as bcpool,
            tc.tile_pool(name="A", bufs=abufs) as apool,
            tc.tile_pool(name="outsb", bufs=obufs) as opool,
            tc.tile_pool(name="psum", bufs=psbufs, space="PSUM") as pspool,
        ):
            # constant column-index tile J[p, j] = j
            j_i = cpool.tile([128, awidth], i32)
            nc.gpsimd.iota(j_i[:], pattern=[[1, awidth]], base=0, channel_multiplier=0)
            j_f = cpool.tile([128, awidth], f32)
            nc.vector.tensor_copy(j_f[:], j_i[:])
            if any_empty_m or stage < 3:
                zeros = cpool.tile([128, D], f32)
                nc.vector.memset(zeros[:], 0.0)
            econst = None
            if nodma:
                econst = []
                for h in range(2):
                    tt = cpool.tile([128, 4 * D], f32r, tag=f"Ec{h}")
                    nc.vector.memset(tt[:].bitcast(f32), 0.5)
                    econst.append(tt)

            last_at = None
            for _ in range(repeat):
                if stage >= 0:
                    av = bcpool.tile([128, RPC * KT * 2], f32, tag="av")
                    nc.sync.dma_start(av[:], av_in[:, :])

                for r in range(RPC):
                    # E row in two batched DMAs of 4 k-tiles each
                    et = []
                    if nodma:
                        for k4 in range(KT):
                            et.append(econst[k4 // 4][:, (k4 % 4) * D : (k4 % 4 + 1) * D])
                    else:
                        for h in range(2):
                            t = epool.tile([128, 4 * D], f32r, tag="E")
                            src = E_in[r, h * 512 : (h + 1) * 512, :].rearrange(
                                "(k p) d -> p k d", p=128
                            )
                            nc.sync.dma_start(
                                t[:].rearrange("p (k d) -> p k d", d=D), src
                            )
                            for k4 in range(4):
                                et.append(t[:, k4 * D : (k4 + 1) * D])

                    # one-hot A windows, one fused DVE op per k-tile
                    ak = {}
                    for k in range(KT if stage >= 1 else 0):
                        w = win(r, k)
                        if w is None:
                            continue
                        j0, wd = w
                        c = (r * KT + k) * 2
                        at = apool.tile([128, awidth], f32r, tag="A")
                        nc.vector.tensor_scalar(
                            at[:, :wd],
                            j_f[:, :wd],
                            av[:, c : c + 1],
                            av[:, c + 1 : c + 2],
                            OP.is_equal,
                            OP.mult,
                        )
                        ak[k] = (at, j0)
                        last_at = at

                    for m in range(MT):
                        if kl[r][m] is None or stage < 2:
                            if outdma:
                                nc.sync.dma_start(
                                    out[r, m * 128 : (m + 1) * 128, :], zeros[:]
                                )
                            continue
                        klo, khi = kl[r][m]
                        ps = pspool.tile([128, D], f32, tag="ps")
                        for k in range(klo, khi):
                            at, j0 = ak[k]
                            lhsT = at[:, m * 128 - j0 : (m + 1) * 128 - j0]
                            first = k == klo
                            last = k == khi - 1
                            for n0 in range(0, D, 512):
                                n1 = min(n0 + 512, D)
                                nc.tensor.matmul(
                                    ps[:, n0:n1],
                                    lhsT,
                                    et[k][:, n0:n1],
                                    start=first,
                                    stop=last,
                                )
                        if stage < 3:
                            if outdma:
                                nc.sync.dma_start(
                                    out[r, m * 128 : (m + 1) * 128, :], zeros[:]
                                )
                            continue
                        osb = opool.tile([128, D], f32, tag="osb")
                        if drain == "act":
                            nc.scalar.activation(osb[:], ps[:], AF.Copy)
                        else:
                            nc.vector.tensor_copy(osb[:], ps[:])
                        if outdma:
                            nc.sync.dma_start(
                                out[r, m * 128 : (m + 1) * 128, :], osb[:]
                            )

            if tok is not None:
                if last_at is not None:
                    nc.sync.dma_start(tok[:], last_at[:, :16].bitcast(f32))
                else:
                    nc.sync.dma_start(tok[:], zeros[:, :16])

    nc.compile()
    return nc


def _prep(bert_embedding, x_bert_offset, x_mask):
    st = x_bert_offset[..., 0].astype(np.int64)
    ed = x_bert_offset[..., 1].astype(np.int64)
    length = ed - st
    valid = (x_mask > 0) & (length > 0)
    scale = np.where(
        valid, 1.0 / np.maximum(length, 1).astype(np.float64), 0.0
    ).astype(np.float32)
    st_ext = np.concatenate([st, ed[:, -1:]], axis=1)  # [B, W+1]

    # word index of each position (-1 if uncovered)
    word_of = np.full((B, S), -1, dtype=np.int64)
    s_idx = np.arange(S)
    for b in range(B):
        j = np.searchsorted(st_ext[b], s_idx, side="right") - 1
        ok = (j >= 0) & (j < W)
        word_of[b] = np.where(ok, j, -1)

    pairs = _active_pairs(st, ed)
    kl, mw = pairs

    E = np.ascontiguousarray(bert_embedding, dtype=np.float32)
    in_maps = []
    for c in range(NCORES):
        av = np.zeros((128, RPC * KT * 2), dtype=np.float32)
        for r in range(RPC):
            b = c * RPC + r
            for k in range(KT):
                if mw[r][k] is None:
                    continue
                j0 = mw[r][k][0] * 128
                col = (r * KT + k) * 2
                s = k * 128 + np.arange(128)
                wj = word_of[b, s]
                covered = wj >= 0
                # window hull guarantees covered words lie inside [j0, j0+wd)
                av[:, col] = np.where(covered, wj - j0, -1).astype(np.float32)
                av[:, col + 1] = np.where(
                    covered, scale[b, np.clip(wj, 0, W - 1)], 0.0
                )
        in_maps.append(
            {
                "E_in": E[c * RPC : (c + 1) * RPC],
                "av_in": av,
            }
        )
    return pairs, in_maps


def kernel(bert_embedding, x_bert_offset, x_mask):
    from concourse.bass_utils import run_bass_kernel_spmd

    bert_embedding = np.asarray(bert_embedding, dtype=np.float32)
    x_bert_offset = np.asarray(x_bert_offset)
    x_mask = np.asarray(x_mask)
    pairs, in_maps = _prep(bert_embedding, x_bert_offset, x_mask)
    key = repr(pairs)
    nc = _CACHE.get(key)
    if nc is None:
        nc = build_program(pairs)
        _CACHE[key] = nc
    res = run_bass_kernel_spmd(nc, in_maps, list(range(NCORES)))
    out = np.concatenate([res.results[c]["out"] for c in range(NCORES)], axis=0)
    return out.astype(np.float32)



# revision 7
# speedup vs baseline: 1.5784x; 1.5784x over previous
"""Trainium2 Bass kernel for ragged subword mean pooling (nn_Bert).

Problem: out[b, j] = mean(bert_embedding[b, st_j:ed_j]) if (mask & ed>st) else 0
Shapes: bert_embedding [32, 1024, 768] f32, x_bert_offset [32, 768, 2] i32,
        x_mask [32, 768] i32 -> out [32, 768, 768] f32.

Strategy (pure data parallel, 4 batch rows per core on 8 cores):
  Spans are contiguous sorted segments, so per row the pooling is
  out = A.T @ E where A[s, j] = scale_j iff st_j <= s < ed_j
  (scale_j = valid/len folds the mean and mask directly into A).
  Each position s belongs to at most ONE word, so every A tile has at
  most one nonzero per partition row. The host ships just that
  (column, value) pair per position and the device reconstructs each
  [128, win] A window in a single fused DVE op against a constant
  column-index tile J:
      A[p, j] = (J[p, j] == idx_p) * val_p

  The kernel is memory-bound, so bytes are minimized two ways:
  1. 16-bit I/O: E is shipped as fp16 (host casts; ~5e-4 rel rounding)
     and the output is written back as fp16 and upcast on the host.
  2. Output compaction: ~37% of words are invalid (masked or empty
     span) and produce zero rows. The word axis is compacted to valid
     words only before building A; the device writes only
     ceil(max_valid/128) m-tiles per row and the host scatters rows
     back to the full [W] axis (invalid rows are zeros).

  The contraction runs on the PE in fp16 (full rate, f32 PSUM
  accumulate). PSUM is drained by alternating scalar/vector copies.
  Only (m, k) tile pairs whose word/position ranges intersect are
  computed; the active-pair hull is derived on the host from the
  actual offsets (a superset is always correct since A is 0 outside).
"""

import sys

if "/opt/trn_rl_repo" not in sys.path:
    sys.path.insert(0, "/opt/trn_rl_repo")

import numpy as np

B, S, W, D = 32, 1024, 768, 768
NCORES = 8
RPC = B // NCORES  # rows per core
KT = S // 128  # 8 k-tiles (positions)

_CACHE = {}


def build_program(pairs, repeat=1, drain="alt", io="ext", stage=3, nodma=False,
                  ebufs=7, abufs=8, psbufs=3, obufs=6, avbufs=2):
    """Build the SPMD Bass program (one program, run on all 8 cores)."""
    import concourse.tile as tile
    from concourse import bacc, mybir

    kl, mw, mtiles = pairs
    MTC = max(mtiles)
    f16 = mybir.dt.float16
    f32 = mybir.dt.float32
    i32 = mybir.dt.int32
    AF = mybir.ActivationFunctionType
    OP = mybir.AluOpType

    nc = bacc.Bacc(
        "TRN2", target_bir_lowering=False, debug=False, num_devices=NCORES
    )

    E_in = nc.dram_tensor("E_in", [RPC, S, D], f16, kind="ExternalInput").ap()
    # packed per (r, k): column 2*(r*KT+k) = one-hot column index within the
    # A window (or -1), column +1 = A value (scale of the word at that
    # position, 0 if masked/empty/uncovered)
    av_in = nc.dram_tensor("av_in", [128, RPC * KT * 2], f32, kind="ExternalInput").ap()
    if io == "ext":
        out = nc.dram_tensor("out", [RPC, MTC * 128, D], f16, kind="ExternalOutput").ap()
        tok = None
    else:
        out = nc.dram_tensor("out_scratch", [RPC, MTC * 128, D], f16).ap()
        tok = nc.dram_tensor("tok", [128, 16], f32, kind="ExternalOutput").ap()
    outdma = not nodma

    def win(r, k):
        if mw[r][k] is None:
            return None
        mlo, mhi = mw[r][k]
        return mlo * 128, (mhi - mlo) * 128

    awidth = 128
    for r in range(RPC):
        for k in range(KT):
            if mw[r][k]:
                awidth = max(awidth, (mw[r][k][1] - mw[r][k][0]) * 128)

    any_empty_m = any(
        kl[r][m] is None for r in range(RPC) for m in range(mtiles[r])
    )

    with tile.TileContext(nc) as tc:
        with (
            tc.tile_pool(name="const", bufs=1) as cpool,
            tc.tile_pool(name="E", bufs=ebufs) as epool,
            tc.tile_pool(name="bc", bufs=avbufs) as bcpool,
            tc.tile_pool(name="A", bufs=abufs) as apool,
            tc.tile_pool(name="outsb", bufs=obufs) as opool,
            tc.tile_pool(name="psum", bufs=psbufs, space="PSUM") as pspool,
        ):
            # constant column-index tile J[p, j] = j (fp16 ints exact to 2048)
            j_i = cpool.tile([128, awidth], i32)
            nc.gpsimd.iota(j_i[:], pattern=[[1, awidth]], base=0, channel_multiplier=0)
            j_f = cpool.tile([128, awidth], f32)
            nc.vector.tensor_copy(j_f[:], j_i[:])
            if any_empty_m or stage < 3:
                zeros = cpool.tile([128, D], f16)
                nc.vector.memset(zeros[:], 0.0)
            econst = None
            if nodma:
                econst = []
                for h in range(2):
                    tt = cpool.tile([128, 4 * D], f16, tag=f"Ec{h}")
                    nc.vector.memset(tt[:], 0.5)
                    econst.append(tt)

            last_at = None
            ndrain = 0
            for _ in range(repeat):
                if stage >= 0:
                    av = bcpool.tile([128, RPC * KT * 2], f32, tag="av")
                    nc.sync.dma_start(av[:], av_in[:, :])

                for r in range(RPC):
                    # E row in two batched DMAs of 4 k-tiles each
                    et = []
                    if nodma:
                        for k4 in range(KT):
                            et.append(econst[k4 // 4][:, (k4 % 4) * D : (k4 % 4 + 1) * D])
                    else:
                        for h in range(2):
                            t = epool.tile([128, 4 * D], f16, tag="E")
                            src = E_in[r, h * 512 : (h + 1) * 512, :].rearrange(
                                "(k p) d -> p k d", p=128
                            )
                            nc.sync.dma_start(
                                t[:].rearrange("p (k d) -> p k d", d=D), src
                            )
                            for k4 in range(4):
                                et.append(t[:, k4 * D : (k4 + 1) * D])

                    # one-hot A windows, one fused DVE op per k-tile
                    ak = {}
                    for k in range(KT if stage >= 1 else 0):
                        w = win(r, k)
                        if w is None:
                            continue
                        j0, wd = w
                        c = (r * KT + k) * 2
                        at = apool.tile([128, awidth], f16, tag="A")
                        nc.vector.tensor_scalar(
                            at[:, :wd],
                            j_f[:, :wd],
                            av[:, c : c + 1],
                            av[:, c + 1 : c + 2],
                            OP.is_equal,
                            OP.mult,
                        )
                        ak[k] = (at, j0)
                        last_at = at

                    for m in range(mtiles[r]):
                        if kl[r][m] is None or stage < 2:
                            if outdma:
                                nc.sync.dma_start(
                                    out[r, m * 128 : (m + 1) * 128, :], zeros[:]
                                )
                            continue
                        klo, khi = kl[r][m]
                        ps = pspool.tile([128, D], f32, tag="ps")
                        for k in range(klo, khi):
                            at, j0 = ak[k]
                            lhsT = at[:, m * 128 - j0 : (m + 1) * 128 - j0]
                            first = k == klo
                            last = k == khi - 1
                            for n0 in range(0, D, 512):
                                n1 = min(n0 + 512, D)
                                nc.tensor.matmul(
                                    ps[:, n0:n1],
                                    lhsT,
                                    et[k][:, n0:n1],
                                    start=first,
                                    stop=last,
                                )
                        if stage < 3:
                            if outdma:
                                nc.sync.dma_start(
                                    out[r, m * 128 : (m + 1) * 128, :], zeros[:]
                                )
                            continue
                        osb = opool.tile([128, D], f16, tag="osb")
                        use_act = drain == "act" or (drain == "alt" and ndrain % 2 == 0)
                        ndrain += 1
                        if use_act:
                            nc.scalar.activation(osb[:], ps[:], AF.Copy)
                        else:
                            nc.vector.tensor_copy(osb[:], ps[:])
                        if outdma:
                            nc.sync.dma_start(
                                out[r, m * 128 : (m + 1) * 128, :], osb[:]
                            )

            if tok is not None:
                if last_at is not None:
                    nc.sync.dma_start(tok[:, :8], last_at[:, :16].bitcast(f32))
                else:
                    nc.sync.dma_start(tok[:, :8], zeros[:, :16].bitcast(f32))

    nc.compile()
    return nc


def _prep_full(bert_embedding, x_bert_offset, x_mask):
    st = x_bert_offset[..., 0].astype(np.int64)
    ed = x_bert_offset[..., 1].astype(np.int64)
    length = ed - st
    valid = (x_mask > 0) & (length > 0)  # [B, W]
    scale = np.where(
        valid, 1.0 / np.maximum(length, 1).astype(np.float64), 0.0
    ).astype(np.float32)

    # compact word axis: keep only valid words
    cidx = np.where(valid, np.cumsum(valid, axis=1) - 1, -1)  # [B, W]
    nv = valid.sum(axis=1).astype(np.int64)  # [B]

    # word index of each position (-1 if uncovered), then compacted
    st_ext = np.concatenate([st, ed[:, -1:]], axis=1)  # [B, W+1]
    word_of = np.full((B, S), -1, dtype=np.int64)
    s_idx = np.arange(S)
    for b in range(B):
        j = np.searchsorted(st_ext[b], s_idx, side="right") - 1
        ok = (j >= 0) & (j < W)
        word_of[b] = np.where(ok, j, -1)
    wsafe = np.clip(word_of, 0, W - 1)
    covered = word_of >= 0
    bidx = np.arange(B)[:, None]
    cword_of = np.where(covered & valid[bidx, wsafe], cidx[bidx, wsafe], -1)  # [B, S]
    cscale = np.where(cword_of >= 0, scale[bidx, wsafe], 0.0).astype(np.float32)

    # per row-slot r: number of m-tiles = max over cores of ceil(nv/128)
    mtiles = []
    for r in range(RPC):
        mt = 1
        for c in range(NCORES):
            mt = max(mt, int(-(-nv[c * RPC + r] // 128)))
        mtiles.append(mt)

    # kl[r][m]: hull of active k-tiles per compacted m-tile, unioned over cores
    kl = []
    for r in range(RPC):
        per_m = []
        for m in range(mtiles[r]):
            klo, khi = KT, 0
            for c in range(NCORES):
                b = c * RPC + r
                sel = (cword_of[b] >= m * 128) & (cword_of[b] < (m + 1) * 128)
                if sel.any():
                    ss = np.nonzero(sel)[0]
                    klo = min(klo, int(ss[0]) // 128)
                    khi = max(khi, int(ss[-1]) // 128 + 1)
            per_m.append((klo, khi) if khi > klo else None)
        kl.append(per_m)

    # mw[r][k]: hull of m-tiles whose kl-range contains k (guarantees every
    # matmul slice lies inside the built A window)
    mw = []
    for r in range(RPC):
        per_k = []
        for k in range(KT):
            mlo, mhi = mtiles[r], 0
            for m in range(mtiles[r]):
                if kl[r][m] and kl[r][m][0] <= k < kl[r][m][1]:
                    mlo = min(mlo, m)
                    mhi = max(mhi, m + 1)
            per_k.append((mlo, mhi) if mhi > mlo else None)
        mw.append(per_k)

    pairs = (kl, mw, tuple(mtiles))

    E16 = np.ascontiguousarray(bert_embedding.astype(np.float16))
    in_maps = []
    for c in range(NCORES):
        av = np.zeros((128, RPC * KT * 2), dtype=np.float32)
        for r in range(RPC):
            b = c * RPC + r
            for k in range(KT):
                if mw[r][k] is None:
                    continue
                j0 = mw[r][k][0] * 128
                col = (r * KT + k) * 2
                s = k * 128 + np.arange(128)
                cw = cword_of[b, s]
                cov = cw >= 0
                # window hull guarantees covered words lie inside [j0, j0+wd)
                av[:, col] = np.where(cov, cw - j0, -1).astype(np.float32)
                av[:, col + 1] = cscale[b, s].astype(np.float32)
        in_maps.append(
            {
                "E_in": E16[c * RPC : (c + 1) * RPC],
                "av_in": av,
            }
        )
    scatter = (valid, cidx, nv)
    return pairs, in_maps, scatter


def _prep(bert_embedding, x_bert_offset, x_mask):
    pairs, in_maps, _ = _prep_full(bert_embedding, x_bert_offset, x_mask)
    return pairs, in_maps


def kernel(bert_embedding, x_bert_offset, x_mask):
    from concourse.bass_utils import run_bass_kernel_spmd

    bert_embedding = np.asarray(bert_embedding, dtype=np.float32)
    x_bert_offset = np.asarray(x_bert_offset)
    x_mask = np.asarray(x_mask)
    pairs, in_maps, scatter = _prep_full(bert_embedding, x_bert_offset, x_mask)
    key = repr(pairs)
    nc = _CACHE.get(key)
    if nc is None:
        nc = build_program(pairs)
        _CACHE[key] = nc
    res = run_bass_kernel_spmd(nc, in_maps, list(range(NCORES)))
    comp = np.concatenate(
        [np.asarray(res.results[c]["out"]) for c in range(NCORES)], axis=0
    )  # [B, MTC*128, D] fp16, compacted word rows

    valid, cidx, nv = scatter
    gather_idx = np.clip(cidx, 0, comp.shape[1] - 1)[..., None]  # [B, W, 1]
    full = np.take_along_axis(
        comp.astype(np.float32), np.broadcast_to(gather_idx, (B, W, D)), axis=1
    )
    full = np.where(valid[..., None], full, np.float32(0.0))
    return np.ascontiguousarray(full, dtype=np.float32)


# revision 18
# speedup vs baseline: 2.0980x; 1.3292x over previous
"""Trainium2 Bass kernel for ragged subword mean pooling (nn_Bert).

Problem: out[b, j] = mean(bert_embedding[b, st_j:ed_j]) if (mask & ed>st) else 0
Shapes: bert_embedding [32, 1024, 768] f32, x_bert_offset [32, 768, 2] i32,
        x_mask [32, 768] i32 -> out [32, 768, 768] f32.

Strategy (pure data parallel, 4 batch rows per core on 8 cores):
  Spans are contiguous sorted segments, so per row the pooling is
  out = A.T @ E where A[s, j] = scale_j iff st_j <= s < ed_j
  (scale_j = valid/len folds the mean and mask directly into A).
  Each position s belongs to at most ONE word, so every A tile has at
  most one nonzero per partition row. The host ships just that
  (column, value) pair per position and the device reconstructs each
  [128, win] A window in a single fused DVE op against a constant
  column-index tile J:
      A[p, j] = (J[p, j] == idx_p) * val_p

  The kernel is memory-bound, so bytes are minimized two ways:
  1. 16-bit I/O: E is shipped as fp16 (host casts; ~5e-4 rel rounding)
     and the output is written back as fp16 and upcast on the host.
  2. Output compaction: ~37% of words are invalid (masked or empty
     span) and produce zero rows. The word axis is compacted to valid
     words only before building A; the device writes only
     ceil(max_valid/128) m-tiles per row and the host scatters rows
     back to the full [W] axis (invalid rows are zeros).

  The contraction runs on the PE in fp16 (full rate, f32 PSUM
  accumulate). PSUM is drained by alternating scalar/vector copies.
  Only (m, k) tile pairs whose word/position ranges intersect are
  computed; the active-pair hull is derived on the host from the
  actual offsets (a superset is always correct since A is 0 outside).
"""

import sys

if "/opt/trn_rl_repo" not in sys.path:
    sys.path.insert(0, "/opt/trn_rl_repo")

import numpy as np

B, S, W, D = 32, 1024, 768, 768
NCORES = 8
RPC = B // NCORES  # rows per core
KT = S // 128  # 8 k-tiles (positions)

_CACHE = {}


def build_program(pairs, repeat=1, drain="alt", io="ext", stage=3, nodma=False,
                  ebufs=7, abufs=8, psbufs=3, obufs=6, avbufs=2,
                  wide_out=False, one_e=False, wq=False):
    """Build the SPMD Bass program (one program, run on all 8 cores)."""
    import concourse.tile as tile
    from concourse import bacc, mybir

    kl, mw, mtiles = pairs[:3]
    maxnv = pairs[3] if len(pairs) > 3 else tuple(m * 128 for m in mtiles)
    MTC = max(mtiles)
    f16 = mybir.dt.float16
    f32 = mybir.dt.float32
    i32 = mybir.dt.int32
    AF = mybir.ActivationFunctionType
    OP = mybir.AluOpType

    nc = bacc.Bacc(
        "TRN2", target_bir_lowering=False, debug=False, num_devices=NCORES
    )

    E_in = nc.dram_tensor("E_in", [RPC, S, D], f16, kind="ExternalInput").ap()
    # packed per (r, k): column 2*(r*KT+k) = one-hot column index within the
    # A window (or -1), column +1 = A value (scale of the word at that
    # position, 0 if masked/empty/uncovered)
    av_in = nc.dram_tensor("av_in", [128, RPC * KT * 2], f32, kind="ExternalInput").ap()
    if io == "ext":
        out = nc.dram_tensor("out", [RPC, MTC * 128, D], f16, kind="ExternalOutput").ap()
        tok = None
    else:
        out = nc.dram_tensor("out_scratch", [RPC, MTC * 128, D], f16).ap()
        tok = nc.dram_tensor("tok", [128, 16], f32, kind="ExternalOutput").ap()
    outdma = not nodma
    wdma = (lambda o, i: nc.scalar.dma_start(o, i)) if wq else (
        lambda o, i: nc.sync.dma_start(o, i)
    )

    def win(r, k):
        if mw[r][k] is None:
            return None
        mlo, mhi = mw[r][k]
        return mlo * 128, (mhi - mlo) * 128

    awidth = 128
    for r in range(RPC):
        for k in range(KT):
            if mw[r][k]:
                awidth = max(awidth, (mw[r][k][1] - mw[r][k][0]) * 128)

    any_empty_m = any(
        kl[r][m] is None for r in range(RPC) for m in range(mtiles[r])
    )

    with tile.TileContext(nc) as tc:
        with (
            tc.tile_pool(name="const", bufs=1) as cpool,
            tc.tile_pool(name="E", bufs=ebufs) as epool,
            tc.tile_pool(name="bc", bufs=avbufs) as bcpool,
            tc.tile_pool(name="A", bufs=abufs) as apool,
            tc.tile_pool(name="outsb", bufs=obufs) as opool,
            tc.tile_pool(name="psum", bufs=psbufs, space="PSUM") as pspool,
        ):
            # constant column-index tile J[p, j] = j (fp16 ints exact to 2048)
            j_i = cpool.tile([128, awidth], i32)
            nc.gpsimd.iota(j_i[:], pattern=[[1, awidth]], base=0, channel_multiplier=0)
            j_f = cpool.tile([128, awidth], f32)
            nc.vector.tensor_copy(j_f[:], j_i[:])
            if any_empty_m or stage < 3:
                zeros = cpool.tile([128, D], f16)
                nc.vector.memset(zeros[:], 0.0)
            econst = None
            if nodma:
                econst = []
                for h in range(2):
                    tt = cpool.tile([128, 4 * D], f16, tag=f"Ec{h}")
                    nc.vector.memset(tt[:], 0.5)
                    econst.append(tt)

            last_at = None
            ndrain = 0
            for _ in range(repeat):
                if stage >= 0:
                    av = bcpool.tile([128, RPC * KT * 2], f32, tag="av")
                    nc.sync.dma_start(av[:], av_in[:, :])

                for r in range(RPC):
                    # E row in two batched DMAs of 4 k-tiles each
                    et = []
                    if nodma:
                        for k4 in range(KT):
                            et.append(econst[k4 // 4][:, (k4 % 4) * D : (k4 % 4 + 1) * D])
                    elif one_e:
                        t = epool.tile([128, 8 * D], f16, tag="E")
                        src = E_in[r, :, :].rearrange("(k p) d -> p k d", p=128)
                        nc.sync.dma_start(
                            t[:].rearrange("p (k d) -> p k d", d=D), src
                        )
                        for k8 in range(8):
                            et.append(t[:, k8 * D : (k8 + 1) * D])
                    else:
                        for h in range(2):
                            t = epool.tile([128, 4 * D], f16, tag="E")
                            src = E_in[r, h * 512 : (h + 1) * 512, :].rearrange(
                                "(k p) d -> p k d", p=128
                            )
                            nc.sync.dma_start(
                                t[:].rearrange("p (k d) -> p k d", d=D), src
                            )
                            for k4 in range(4):
                                et.append(t[:, k4 * D : (k4 + 1) * D])

                    # one-hot A windows, one fused DVE op per k-tile
                    ak = {}
                    for k in range(KT if stage >= 1 else 0):
                        w = win(r, k)
                        if w is None:
                            continue
                        j0, wd = w
                        c = (r * KT + k) * 2
                        at = apool.tile([128, awidth], f16, tag="A")
                        nc.vector.tensor_scalar(
                            at[:, :wd],
                            j_f[:, :wd],
                            av[:, c : c + 1],
                            av[:, c + 1 : c + 2],
                            OP.is_equal,
                            OP.mult,
                        )
                        ak[k] = (at, j0)
                        last_at = at

                    mt = mtiles[r]
                    wosb = None
                    if wide_out and stage >= 3:
                        wosb = opool.tile([128, mt * D], f16, tag="wosb")
                    for m in range(mt):
                        if kl[r][m] is None or stage < 2:
                            if outdma and wosb is None:
                                wdma(
                                    out[r, m * 128 : (m + 1) * 128, :], zeros[:]
                                )
                            elif wosb is not None:
                                nc.vector.memset(wosb[:, m * D : (m + 1) * D], 0.0)
                            continue
                        klo, khi = kl[r][m]
                        ps = pspool.tile([128, D], f32, tag="ps")
                        for k in range(klo, khi):
                            at, j0 = ak[k]
                            lhsT = at[:, m * 128 - j0 : (m + 1) * 128 - j0]
                            first = k == klo
                            last = k == khi - 1
                            for n0 in range(0, D, 512):
                                n1 = min(n0 + 512, D)
                                nc.tensor.matmul(
                                    ps[:, n0:n1],
                                    lhsT,
                                    et[k][:, n0:n1],
                                    start=first,
                                    stop=last,
                                )
                        if stage < 3:
                            if outdma:
                                wdma(
                                    out[r, m * 128 : (m + 1) * 128, :], zeros[:]
                                )
                            continue
                        if wosb is not None:
                            osb = wosb[:, m * D : (m + 1) * D]
                        else:
                            osbt = opool.tile([128, D], f16, tag="osb")
                            osb = osbt[:]
                        use_act = drain == "act" or (drain == "alt" and ndrain % 2 == 0)
                        ndrain += 1
                        if use_act:
                            nc.scalar.activation(osb, ps[:], AF.Copy)
                        else:
                            nc.vector.tensor_copy(osb, ps[:])
                        if outdma and wosb is None:
                            hi = min((m + 1) * 128, maxnv[r])
                            rows = hi - m * 128
                            if rows > 0:
                                wdma(
                                    out[r, m * 128 : hi, :],
                                    osb[:rows] if rows < 128 else osb,
                                )
                    if outdma and wosb is not None:
                        full_mt = maxnv[r] // 128
                        if full_mt:
                            wdma(
                                out[r, : full_mt * 128, :].rearrange(
                                    "(m p) d -> p m d", p=128
                                ),
                                wosb[:, : full_mt * D].rearrange(
                                    "p (m d) -> p m d", d=D
                                ),
                            )
                        rows = maxnv[r] - full_mt * 128
                        if rows:
                            wdma(
                                out[r, full_mt * 128 : maxnv[r], :],
                                wosb[:rows, full_mt * D : (full_mt + 1) * D],
                            )

            if tok is not None:
                if last_at is not None:
                    nc.sync.dma_start(tok[:, :8], last_at[:, :16].bitcast(f32))
                else:
                    nc.sync.dma_start(tok[:, :8], zeros[:, :16].bitcast(f32))

    nc.compile()
    return nc


def _prep_full(bert_embedding, x_bert_offset, x_mask):
    st = x_bert_offset[..., 0].astype(np.int64)
    ed = x_bert_offset[..., 1].astype(np.int64)
    length = ed - st
    valid = (x_mask > 0) & (length > 0)  # [B, W]
    scale = np.where(
        valid, 1.0 / np.maximum(length, 1).astype(np.float64), 0.0
    ).astype(np.float32)

    # compact word axis: keep only valid words
    cidx = np.where(valid, np.cumsum(valid, axis=1) - 1, -1)  # [B, W]
    nv = valid.sum(axis=1).astype(np.int64)  # [B]

    # word index of each position (-1 if uncovered), then compacted
    st_ext = np.concatenate([st, ed[:, -1:]], axis=1)  # [B, W+1]
    word_of = np.full((B, S), -1, dtype=np.int64)
    s_idx = np.arange(S)
    for b in range(B):
        j = np.searchsorted(st_ext[b], s_idx, side="right") - 1
        ok = (j >= 0) & (j < W)
        word_of[b] = np.where(ok, j, -1)
    wsafe = np.clip(word_of, 0, W - 1)
    covered = word_of >= 0
    bidx = np.arange(B)[:, None]
    cword_of = np.where(covered & valid[bidx, wsafe], cidx[bidx, wsafe], -1)  # [B, S]
    cscale = np.where(cword_of >= 0, scale[bidx, wsafe], 0.0).astype(np.float32)

    # per row-slot r: number of m-tiles = max over cores of ceil(nv/128)
    mtiles = []
    for r in range(RPC):
        mt = 1
        for c in range(NCORES):
            mt = max(mt, int(-(-nv[c * RPC + r] // 128)))
        mtiles.append(mt)

    # kl[r][m]: hull of active k-tiles per compacted m-tile, unioned over cores
    kl = []
    for r in range(RPC):
        per_m = []
        for m in range(mtiles[r]):
            klo, khi = KT, 0
            for c in range(NCORES):
                b = c * RPC + r
                sel = (cword_of[b] >= m * 128) & (cword_of[b] < (m + 1) * 128)
                if sel.any():
                    ss = np.nonzero(sel)[0]
                    klo = min(klo, int(ss[0]) // 128)
                    khi = max(khi, int(ss[-1]) // 128 + 1)
            per_m.append((klo, khi) if khi > klo else None)
        kl.append(per_m)

    # mw[r][k]: hull of m-tiles whose kl-range contains k (guarantees every
    # matmul slice lies inside the built A window)
    mw = []
    for r in range(RPC):
        per_k = []
        for k in range(KT):
            mlo, mhi = mtiles[r], 0
            for m in range(mtiles[r]):
                if kl[r][m] and kl[r][m][0] <= k < kl[r][m][1]:
                    mlo = min(mlo, m)
                    mhi = max(mhi, m + 1)
            per_k.append((mlo, mhi) if mhi > mlo else None)
        mw.append(per_k)

    maxnv = tuple(
        max(int(nv[c * RPC + r]) for c in range(NCORES)) for r in range(RPC)
    )
    pairs = (kl, mw, tuple(mtiles), maxnv)

    E16 = np.ascontiguousarray(bert_embedding.astype(np.float16))
    in_maps = []
    for c in range(NCORES):
        av = np.zeros((128, RPC * KT * 2), dtype=np.float32)
        for r in range(RPC):
            b = c * RPC + r
            for k in range(KT):
                if mw[r][k] is None:
                    continue
                j0 = mw[r][k][0] * 128
                col = (r * KT + k) * 2
                s = k * 128 + np.arange(128)
                cw = cword_of[b, s]
                cov = cw >= 0
                # window hull guarantees covered words lie inside [j0, j0+wd)
                av[:, col] = np.where(cov, cw - j0, -1).astype(np.float32)
                av[:, col + 1] = cscale[b, s].astype(np.float32)
        in_maps.append(
            {
                "E_in": E16[c * RPC : (c + 1) * RPC],
                "av_in": av,
            }
        )
    scatter = (valid, cidx, nv)
    return pairs, in_maps, scatter


def _prep(bert_embedding, x_bert_offset, x_mask):
    pairs, in_maps, _ = _prep_full(bert_embedding, x_bert_offset, x_mask)
    return pairs, in_maps


def kernel(bert_embedding, x_bert_offset, x_mask):
    from concourse.bass_utils import run_bass_kernel_spmd

    bert_embedding = np.asarray(bert_embedding, dtype=np.float32)
    x_bert_offset = np.asarray(x_bert_offset)
    x_mask = np.asarray(x_mask)
    pairs, in_maps, scatter = _prep_full(bert_embedding, x_bert_offset, x_mask)
    key = repr(pairs)
    nc = _CACHE.get(key)
    if nc is None:
        nc = build_program(pairs)
        _CACHE[key] = nc
    res = run_bass_kernel_spmd(nc, in_maps, list(range(NCORES)))
    comp = np.concatenate(
        [np.asarray(res.results[c]["out"]) for c in range(NCORES)], axis=0
    )  # [B, MTC*128, D] fp16, compacted word rows

    valid, cidx, nv = scatter
    gather_idx = np.clip(cidx, 0, comp.shape[1] - 1)[..., None]  # [B, W, 1]
    full = np.take_along_axis(
        comp.astype(np.float32), np.broadcast_to(gather_idx, (B, W, D)), axis=1
    )
    full = np.where(valid[..., None], full, np.float32(0.0))
    return np.ascontiguousarray(full, dtype=np.float32)
